# revision 107
# baseline (speedup 1.0000x reference)
"""Causal self-attention with RoPE, tensor-parallel over heads on 8 TRN2 NeuronCores.

Model (from the reference):
    q/k/v = x @ W{q,k,v}.T          x: (1, 2048, 2048), 16 heads x 128 head_dim
    rope(q), rope(k)                half-rotation, 32 nonzero freqs
    causal softmax(q k^T / sqrt(128)) @ v
    out = (y / 3) @ Wo.T

Sharding: 2 heads per core. Each core computes its heads' q/k/v projections,
attention, and a partial c_proj (its 256 columns of the hd contraction);
the host sums the 8 partial outputs (the "all-reduce after c_proj").

Per-core kernel layout choices:
  - Everything transposed so the contraction dim is always on partitions:
    host supplies x in fp8 DoubleRow k-subtile layouts plus pre-transposed,
    pre-quantized weight slices.
  - q/k projections in fp8e4 DoubleRow perf mode (two 128-row k-subtiles
    contracted per pass): weights pre-scaled by 64 on host; the 64*64 factor
    is folded into the softmax exp scale.
  - v projection: 3-pass split-fp8 DoubleRow (x8*Wva + xlo*Wvb + x8*Wvc).
  - Scores in fp8 DoubleRow via the SPLIT-PAIR trick: the DR pair slots
    carry (k8, klo) where klo = f8(k_f16 - k8), against (q8, q8); the
    contraction computes (k8+klo)*q8 = k_f16-precision * q8 at 0.5 cyc/col.
  - c_proj in fp8 DoubleRow the same way: slots (y8, ylo) x (W8', W8') give
    y at f16 precision vs W8', plus one head-paired (y8 x W8res) residual
    pass; 3 DR passes replace 4 f16-rate passes.
  - RoPE via an SBUF->SBUF partition-rotating DMA (roll-by-64) + 3 DVE ops;
    roped q is written straight to fp8 (it only feeds the score matmul).
  - Softmax without max-subtraction; denominator via DVE f16 vecsum
    accumulation + all-ones matmuls; probabilities stay f16 for the PV
    matmul (fp8 probs would breach the error budget).
  - Quantize/duplicate ops for the split pairs run on the otherwise-idle
    GPSIMD engine.
  - Output partials in fp16 (halves the output DMA; host sums in float64).
"""

import numpy as np

T = 2048
D = 2048
H = 16
DH = 128
N_CORES = 8
H_LOC = H // N_CORES          # heads per core = 2
HD_LOC = H_LOC * DH           # local head dims = 256
TCH = 512                     # query-chunk width
N_CH = T // TCH               # 4 chunks
KO = D // 128                 # 16 contraction subtiles
KO2 = KO // 2                 # 8 DoubleRow k-subtile pairs
WS = 64.0                     # host prescale on Wq/Wk before fp8 quantization
SCALE = (DH ** 0.5) / DH      # 1/sqrt(128)
YS = 16.0                     # on-chip prescale on y before fp8 split
BETA = 256.0                  # host prescale on Wo before fp8 split
OSC = 1.0 / (YS * BETA)       # c_proj psum evacuation scale

_CACHE = {}


def build_program():
    """Build (once) the single-core Bass program shared by all 8 cores."""
    if "nc" in _CACHE:
        return _CACHE["nc"]

    from contextlib import ExitStack

    import concourse.bacc as bacc
    import concourse.mybir as mybir
    import concourse.tile as tile

    f32 = mybir.dt.float32
    bf16 = mybir.dt.bfloat16
    f16 = mybir.dt.float16
    f8 = mybir.dt.float8e4
    EXP = mybir.ActivationFunctionType.Exp
    DR = mybir.MatmulPerfMode.DoubleRow

    nc = bacc.Bacc("TRN2", target_bir_lowering=False)

    x8_d = nc.dram_tensor("x8", (128, KO2, 2, T), f8, kind="ExternalInput")
    xl_d = nc.dram_tensor("x8lo", (128, KO2, 2, T), f8, kind="ExternalInput")
    wq_d = nc.dram_tensor("wq8", (128, KO2, 2, HD_LOC), f8, kind="ExternalInput")
    wk_d = nc.dram_tensor("wk8", (128, KO2, 2, HD_LOC), f8, kind="ExternalInput")
    wva_d = nc.dram_tensor("wv8a", (128, KO2, 2, HD_LOC), f8, kind="ExternalInput")
    wvb_d = nc.dram_tensor("wv8b", (128, KO2, 2, HD_LOC), f8, kind="ExternalInput")
    wvc_d = nc.dram_tensor("wv8c", (128, KO2, 2, HD_LOC), f8, kind="ExternalInput")
    wo8p_d = nc.dram_tensor("wo8p", (128, H_LOC, D), f8, kind="ExternalInput")
    wr8_d = nc.dram_tensor("wr8", (128, H_LOC, D), f8, kind="ExternalInput")
    ct_d = nc.dram_tensor("ctab", (128, 1, T), f16, kind="ExternalInput")
    st_d = nc.dram_tensor("stab", (128, 1, T), f16, kind="ExternalInput")
    roll_d = nc.dram_tensor("roll", (128, 128), f16, kind="ExternalInput")
    tri_d = nc.dram_tensor("tri", (128, 1, 128), f16, kind="ExternalInput")
    out_d = nc.dram_tensor("outp", (T, D), f16, kind="ExternalOutput")

    with tile.TileContext(nc) as tc, ExitStack() as ctx:
        persist = ctx.enter_context(tc.tile_pool(name="persist", bufs=1))
        qpool = ctx.enter_context(tc.tile_pool(name="qpool", bufs=2))
        q8pool = ctx.enter_context(tc.tile_pool(name="q8pool", bufs=2))
        ypool = ctx.enter_context(tc.tile_pool(name="ypool", bufs=2))
        y8pool = ctx.enter_context(tc.tile_pool(name="y8pool", bufs=2))
        xpool = ctx.enter_context(tc.tile_pool(name="xpool", bufs=2))
        ptpool = ctx.enter_context(tc.tile_pool(name="ptpool", bufs=8))
        rtmp = ctx.enter_context(tc.tile_pool(name="rtmp", bufs=3))
        rollp = ctx.enter_context(tc.tile_pool(name="rollp", bufs=2))
        spool = ctx.enter_context(tc.tile_pool(name="spool", bufs=2))
        opool = ctx.enter_context(tc.tile_pool(name="opool", bufs=8))
        psum_p = ctx.enter_context(tc.tile_pool(name="psum_p", bufs=2, space="PSUM"))
        psum_mix = ctx.enter_context(tc.tile_pool(name="psum_mix", bufs=2, space="PSUM"))
        psum_ot = ctx.enter_context(tc.tile_pool(name="psum_ot", bufs=2, space="PSUM"))

        def ps_tile(pool=None):
            return (pool or psum_p).tile([128, TCH], f32, tag="ps", name="ps")

        def mix_tile():
            return psum_mix.tile([128, H_LOC, TCH], f32, tag="mix", name="mix")

        # --- resident tensors ---
        w_q = persist.tile([128, KO2, 2, HD_LOC], f8, tag="w_q")
        w_k = persist.tile([128, KO2, 2, HD_LOC], f8, tag="w_k")
        w_va = persist.tile([128, KO2, 2, HD_LOC], f8, tag="w_va")
        w_vb = persist.tile([128, KO2, 2, HD_LOC], f8, tag="w_vb")
        w_vc = persist.tile([128, KO2, 2, HD_LOC], f8, tag="w_vc")
        wo8p = persist.tile([128, H_LOC, D], f8, tag="wo8p")
        wr8 = persist.tile([128, H_LOC, D], f8, tag="wr8")
        kt = persist.tile([128, H_LOC, T], f16, tag="kt")
        k8p = persist.tile([128, H_LOC, 2, T], f8, tag="k8p")
        vt = persist.tile([128, KO, HD_LOC], f16, tag="vt")
        ctab = persist.tile([128, 1, T], f16, tag="ctab")
        stab = persist.tile([128, 1, T], f16, tag="stab")
        roll = persist.tile([128, 128], f16, tag="roll")
        ones = persist.tile([128, 128], f16, tag="ones")
        nc.gpsimd.memset(ones[:], 1.0)
        tri = persist.tile([128, 1, 128], f16, tag="tri")

        def issue_x(c):
            """Queue the x chunk DMAs for chunk c (weights too on chunk 0)."""
            cs = c * TCH
            x8c = xpool.tile([128, KO2, 2, TCH], f8, tag="x8c", name="x8c")
            xloc = xpool.tile([128, KO2, 2, TCH], f8, tag="xloc", name="xloc")
            if c == 0:
                # ordered so the first PE work (q-proj, rope) unblocks
                # soonest; first transfers split+interleaved so the opening
                # matmul group can start after half the data has landed
                # weights ride the ACT queue: at t=0 both SEQs are empty,
                # so the x and w streams dispatch concurrently and the first
                # projection matmuls unblock a transfer earlier
                nc.scalar.dma_start(w_q[:, :2], wq_d[:, :2])
                nc.sync.dma_start(x8c[:, :2], x8_d[:, :2, :, cs:cs + TCH])
                nc.scalar.dma_start(w_q[:, 2:], wq_d[:, 2:])
                nc.sync.dma_start(x8c[:, 2:4], x8_d[:, 2:4, :, cs:cs + TCH])
                nc.scalar.dma_start(w_k[:, :2], wk_d[:, :2])
                nc.scalar.dma_start(w_k[:, 2:], wk_d[:, 2:])
                nc.sync.dma_start(x8c[:, 4:6], x8_d[:, 4:6, :, cs:cs + TCH])
                nc.sync.dma_start(x8c[:, 6:], x8_d[:, 6:, :, cs:cs + TCH])
                # rope tables early: chunk 0's attention can then start while
                # the v weights are still streaming
                nc.sync.dma_start(roll[:], roll_d[:])
                nc.sync.dma_start(ctab[:], ct_d[:])
                nc.sync.dma_start(stab[:], st_d[:])
                nc.sync.dma_start(tri[:], tri_d[:])
                nc.sync.dma_start(w_va[:], wva_d[:])
                nc.sync.dma_start(w_vc[:], wvc_d[:])
                nc.sync.dma_start(xloc[:], xl_d[:, :, :, cs:cs + TCH])
                nc.sync.dma_start(w_vb[:], wvb_d[:])
            elif c == 1:
                nc.sync.dma_start(x8c[:], x8_d[:, :, :, cs:cs + TCH])
                nc.sync.dma_start(xloc[:], xl_d[:, :, :, cs:cs + TCH])
            elif c == 2:
                nc.sync.dma_start(x8c[:], x8_d[:, :, :, cs:cs + TCH])
                nc.sync.dma_start(xloc[:], xl_d[:, :, :, cs:cs + TCH])
                nc.sync.dma_start(wo8p[:], wo8p_d[:])
                nc.sync.dma_start(wr8[:], wr8_d[:])
            else:
                nc.sync.dma_start(x8c[:], x8_d[:, :, :, cs:cs + TCH])
                nc.sync.dma_start(xloc[:], xl_d[:, :, :, cs:cs + TCH])
            return (x8c, xloc)

        def rope_muls(src, cl, cs):
            """RoPE front half: roll-by-64 on PE, psum evacuated to f16 on
            the (otherwise idle) GPSIMD engine, then the table multiplies on
            DVE in all-16-bit 2x mode. Returns (a, b) f16 tmp tiles."""
            rolled = mix_tile()
            for h in range(H_LOC):
                nc.tensor.matmul(rolled[:, h, :], lhsT=roll,
                                 rhs=src[:, h, cl:cl + TCH],
                                 start=True, stop=True)
            a = rtmp.tile([128, H_LOC, TCH], f16, tag="ra", name="ra")
            b = rtmp.tile([128, H_LOC, TCH], f16, tag="rb", name="rb")
            nc.vector.tensor_mul(out=a, in0=src[:, :, cl:cl + TCH],
                                 in1=ctab[:, :, cs:cs + TCH].to_broadcast(
                                     (128, H_LOC, TCH)))
            # read the rolled psum straight into the stab multiply: one DVE
            # op replaces the ACT evacuation + f16 multiply
            nc.vector.tensor_mul(out=b, in0=rolled,
                                 in1=stab[:, :, cs:cs + TCH].to_broadcast(
                                     (128, H_LOC, TCH)))
            return a, b

        def rope_q(qc, q8, cs):
            """RoPE q and quantize: the final add writes fp8 directly (f16
            roped q is never needed; the score rhs broadcasts the pair)."""
            a, b = rope_muls(qc, 0, cs)
            nc.vector.tensor_add(out=q8[:, :, :], in0=a, in1=b)

        def rope_k(cs):
            """RoPE k in place (f16) then split-quantize to (k8, klo) pairs
            on GPSIMD: klo = f8(kt - k8) so (k8+klo) recovers kt exactly."""
            a, b = rope_muls(kt, cs, cs)
            nc.vector.tensor_add(out=kt[:, :, cs:cs + TCH], in0=a, in1=b)
            # k8/klo gate the next span's score matmuls: run the copy on ACT
            # and the sub on DVE (fast engines) rather than GPSIMD
            nc.scalar.copy(out=k8p[:, :, 0, cs:cs + TCH],
                           in_=kt[:, :, cs:cs + TCH])
            nc.gpsimd.tensor_sub(out=k8p[:, :, 1, cs:cs + TCH],
                                 in0=kt[:, :, cs:cs + TCH],
                                 in1=k8p[:, :, 0, cs:cs + TCH])

        def qk_head(w_sb, x8c, dsl, h):
            """One head's q-or-k projection: 16 DoubleRow matmuls + copy."""
            ps = ps_tile()
            for tp in range(2):
                for jko in range(KO2):
                    nc.tensor.matmul(
                        ps[:, tp * 256:(tp + 1) * 256],
                        lhsT=w_sb[:, jko, :, h * 128:(h + 1) * 128],
                        rhs=x8c[:, jko, :, tp * 256:(tp + 1) * 256],
                        start=(jko == 0),
                        stop=(jko == KO2 - 1),
                        perf_mode=DR,
                    )
            nc.scalar.copy(out=dsl, in_=ps)

        def v_half(c, xc, tu):
            """Half a chunk's v projection: 3-pass split-fp8 DoubleRow.

            v = x8*Wva(64w) + xlo(8dx)*Wvb(8w) + x8*Wvc(64dw); PSUM holds
            64*v, the evacuation copy scales by 1/64."""
            x8c, xloc = xc
            passes = ((x8c, w_va), (x8c, w_vc), (xloc, w_vb))
            ps = ps_tile()
            for tt in (2 * tu, 2 * tu + 1):
                sub = ps[:, (tt % 2) * HD_LOC:(tt % 2 + 1) * HD_LOC]
                for pi, (xt, wt) in enumerate(passes):
                    for jko in range(KO2):
                        nc.tensor.matmul(
                            sub,
                            lhsT=xt[:, jko, :, tt * 128:(tt + 1) * 128],
                            rhs=wt[:, jko, :, :],
                            start=(pi == 0 and jko == 0),
                            stop=(pi == 2 and jko == KO2 - 1),
                            perf_mode=DR,
                        )

            gt2 = c * 2 + tu
            nc.vector.tensor_scalar_mul(out=vt[:, 2 * gt2:2 * gt2 + 2, :],
                                        in0=ps, scalar1=1.0 / WS)

        def proj_quanta(c, xc):
            """q projection for chunk c as quanta: [head0, head1, rope].
            Tiles are allocated eagerly so later spans can reference q8."""
            qc = qpool.tile([128, H_LOC, TCH], f16, tag="qc", name="qc")
            q8 = q8pool.tile([128, H_LOC, TCH], f8, tag="q8", name="q8")
            quanta = [
                lambda: qk_head(w_q, xc[0], qc[:, 0, :], 0),
                lambda: qk_head(w_q, xc[0], qc[:, 1, :], 1),
                lambda: rope_q(qc, q8, c * TCH),
            ]
            return quanta, qc, q8

        def proj_q(c, xc, do_rope=True):
            """q projection + its rope/quantize for t-chunk c. The roll
            DMAs are issued per head as soon as that head's evacuation is
            queued (even when the rope itself is deferred)."""
            qc = qpool.tile([128, H_LOC, TCH], f16, tag="qc", name="qc")
            q8 = q8pool.tile([128, H_LOC, TCH], f8, tag="q8", name="q8")
            for h in range(H_LOC):
                qk_head(w_q, xc[0], qc[:, h, :], h)
            if do_rope:
                rope_q(qc, q8, c * TCH)
            return qc, q8

        def kv_quanta(c, xc):
            """k/v projections for chunk c as quanta (PE-heavy, ACT-light) --
            interleaved into the previous chunk's attention span."""
            cs = c * TCH

            def k_head(h):
                qk_head(w_k, xc[0], kt[:, h, cs:cs + TCH], h)

            return [
                lambda: k_head(0),
                lambda: k_head(1),
                lambda: v_half(c, xc, 0),
                lambda: v_half(c, xc, 1),
                lambda: rope_k(cs),
            ]

        def attn_span(q0, W, q8, off, yc, yc8, jt_lo=0, jt_hi=None,
                      state=None, ot_pool=None, filler=(), fill_per_jt=1,
                      fast_quant=False):
            """Causal attention for queries [q0, q0+W), heads interleaved.

            q0 must be 128-aligned; W in {256, 512}. q8 holds the chunk's
            roped fp8 queries (dup pair); off is q0's offset within q8/yc."""
            d0 = q0 // 128          # first diagonal j-tile
            n_jt = d0 + W // 128
            if state is None:
                ots = [ps_tile(ot_pool or psum_ot) for _ in range(H_LOC)]
                vecsums = [spool.tile([128, H_LOC, TCH], f16,
                                      tag=f"vecsum{par}", name="vecsum")
                           for par in range(2)]
            else:
                ots, vecsums = state
            if jt_hi is None:
                jt_hi = n_jt
            filler = iter(filler) if not hasattr(filler, "__next__") else filler
            recipb = rtmp.tile([128, H_LOC, TCH], f16, tag="recipb",
                               name="recipb")
            den_box = [None]
            for jt in range(jt_lo, jt_hi):
                # interleave deferred work (previous chunk's c_proj) into the
                # jt loop: PE's stream is in-order per engine, so this is the
                # only way it can fill the exp-gated gaps between j-tiles
                for _ in range(fill_per_jt):
                    q = next(filler, None)
                    if q is not None:
                        q()
                pair = mix_tile()
                m = jt - d0
                # diagonal block: cols < 128m fully masked -- never written,
                # never read (partial-width ops)
                lo = 128 * m if m > 0 else 0
                for h in range(H_LOC):
                    # split-pair DR: (k8, klo) x broadcast (q8, q8) ->
                    # kt_f16-precision * q8 at 0.5 cyc/col
                    nc.tensor.matmul(
                        pair[:, h, lo:W],
                        lhsT=k8p[:, h, :, jt * 128:(jt + 1) * 128],
                        rhs=q8[:, h:h + 1, off + lo:off + W].to_broadcast(
                            (128, 2, W - lo)),
                        start=True,
                        stop=True,
                        perf_mode=DR,
                    )
                pt = ptpool.tile([128, H_LOC, TCH], f16, tag="pt", name="pt")
                # both heads in ONE activation call (strided AP when lo > 0);
                # q/k carry the 64x host prescale each -> 1/4096 here
                nc.scalar.activation(out=pt[:, :, lo:W], in_=pair[:, :, lo:W],
                                     func=EXP, scale=SCALE / (WS * WS))
                if m >= 0:
                    # mask the diagonal block, both heads in one op
                    nc.vector.tensor_mul(
                        out=pt[:, :, 128 * m:128 * (m + 1)],
                        in0=pt[:, :, 128 * m:128 * (m + 1)],
                        in1=tri[:].to_broadcast((128, H_LOC, 128)),
                    )
                # probability row-sum accumulator (all DVE: f16 runs 2x and
                # GPSIMD's 0.42-efficiency adds would chain on the critical
                # path). jt==0 initializes via copy; on q0=0 spans jt==1 is
                # diagonal with cols < 128 unwritten, so never full-copy there.
                vs = vecsums[0]
                if jt == 0:
                    nc.vector.tensor_copy(out=vs[:, 0, :W], in_=pt[:, 0, :W])
                    nc.gpsimd.tensor_copy(out=vs[:, 1, :W], in_=pt[:, 1, :W])
                else:
                    nc.vector.tensor_add(out=vs[:, 0, lo:W], in0=vs[:, 0, lo:W],
                                         in1=pt[:, 0, lo:W])
                    nc.gpsimd.tensor_add(out=vs[:, 1, lo:W],
                                         in0=vs[:, 1, lo:W],
                                         in1=pt[:, 1, lo:W])
                for h in range(H_LOC):
                    # partial-width diagonal writes skip the (bank-granular)
                    # psum group check -- EXCEPT the last j-tile, whose
                    # stop must be bookkept so the ymul read sees a closed
                    # group
                    nc.tensor.matmul(
                        ots[h][:, lo:W],
                        lhsT=vt[:, jt, h * 128:(h + 1) * 128],
                        rhs=pt[:, h, lo:W],
                        start=(jt == 0),
                        stop=(jt == n_jt - 1),
                        skip_group_check=(lo > 0 and jt != n_jt - 1),
                    )
                if jt == n_jt - 1:
                    # first-half denominator + reciprocal: columns [0, W/2)
                    # of the vecsum are final since jt d0+1, and the score
                    # ring has slack this late in the span
                    hw_ = W // 2
                    den_box[0] = mix_tile()
                    for h in range(H_LOC):
                        nc.tensor.matmul(den_box[0][:, h, :hw_], lhsT=ones,
                                         rhs=vecsums[0][:, h, :hw_],
                                         start=True, stop=True)
                    with nc.allow_low_precision(reason="denominators are "
                                                "O(100); fp16 keeps 3+ "
                                                "digits"):
                        nc.vector.reciprocal(out=recipb[:, :, :hw_],
                                             in_=den_box[0][:, :, :hw_])
            if jt_hi < n_jt:
                return (ots, vecsums), filler
            # denominator second half (the first half was folded into the
            # jt loop two j-tiles early, where its vecsum columns are final)
            hw_ = W // 2
            den = den_box[0]
            for h in range(H_LOC):
                nc.tensor.matmul(den[:, h, hw_:W], lhsT=ones,
                                 rhs=vecsums[0][:, h, hw_:W],
                                 start=True, stop=True)
            # evacuate the PV accumulators to fp16 SBUF on ACT (overlaps the
            # den/recip chain) with the YS prescale for the fp8 y split,
            # then the normalize multiplies run in DVE's all-16-bit 2x mode
            oc = rtmp.tile([128, H_LOC, TCH], f16, tag="oc", name="oc")
            for h in range(H_LOC):
                nc.scalar.mul(out=oc[:, h, :W], in_=ots[h][:, :W], mul=YS)
            with nc.allow_low_precision(reason="denominators are O(100) and "
                                        "fp16 keeps 3+ digits"):
                nc.vector.reciprocal(out=recipb[:, :, hw_:W],
                                     in_=den[:, :, hw_:W])
            for h in range(H_LOC):
                nc.vector.tensor_mul(out=yc[:, h, off:off + W],
                                     in0=oc[:, h, :W], in1=recipb[:, h, :W])
            # split-quantize y for the fp8-DR c_proj: slot0 = f8(YS*y),
            # slot1 = f8(YS*y - slot0); GPSIMD keeps this off ACT/DVE, and
            # per-128-block granularity lets the first c_proj quantum start
            # ~1us after the first normalize instead of behind a full-width
            # copy+sub chain
            for tt in range(W // 128):
                blk = slice(off + tt * 128, off + (tt + 1) * 128)
                if fast_quant:
                    # tail path: ACT/DVE are idle once the last exp retires,
                    # and the slow GPSIMD ops would pace the final c_proj
                    nc.scalar.copy(out=yc8[:, :, 0, blk], in_=yc[:, :, blk])
                    nc.vector.tensor_sub(out=yc8[:, :, 1, blk],
                                         in0=yc[:, :, blk],
                                         in1=yc8[:, :, 0, blk])
                else:
                    nc.gpsimd.tensor_copy(out=yc8[:, :, 0, blk],
                                          in_=yc[:, :, blk])
                    nc.vector.tensor_sub(out=yc8[:, :, 1, blk],
                                         in0=yc[:, :, blk],
                                         in1=yc8[:, :, 0, blk])
            return filler

        def cproj_matmuls(ps_ap, yc8, off, tt, csl):
            """The 3 split-pair DR passes for one [128, 512] output block:
            (y8, ylo) x (W8', W8') per head, then (y8_h0, y8_h1) x W8res."""
            blk = slice(off + tt * 128, off + (tt + 1) * 128)
            for h in range(H_LOC):
                nc.tensor.matmul(
                    ps_ap,
                    lhsT=yc8[:, h, :, blk],
                    rhs=wo8p[:, h:h + 1, csl].to_broadcast(
                        (128, 2, csl.stop - csl.start)),
                    start=(h == 0),
                    stop=False,
                    perf_mode=DR,
                )
            nc.tensor.matmul(
                ps_ap,
                lhsT=yc8[:, :, 0, blk],
                rhs=wr8[:, :, csl],
                start=False,
                stop=True,
                perf_mode=DR,
            )

        def cproj_quanta(q0, W, yc8, off, pools=None, dve_only=False,
                         pool_evac=False, spread_dma=False):
            """Partial c_proj for rows [q0, q0+W) as a list of work quanta.

            Each quantum emits half a 128-row tile (6 DR matmuls + one [128,
            1024] PSUM evacuation + its output DMA); the caller threads them
            into an attention span's jt loop so PE fills exp-gated gaps.
            pools: optional psum pool rotation (tail c_projs run when the
            proj/attention pools are idle -- deeper pipelining)."""
            obs = {}

            def quantum(tt, half, pool):
                gt = q0 // 128 + tt
                if half == 0:
                    obs[tt] = opool.tile([128, D], f16, tag="ob", name="ob")
                if pool is None:
                    ps = mix_tile()
                else:
                    ps = pool.tile([128, H_LOC, TCH], f32, tag="cp", name="cp")
                for nk in range(2):
                    nck = half * 2 + nk
                    cproj_matmuls(ps[:, nk, :], yc8, off, tt,
                                  slice(nck * 512, (nck + 1) * 512))
                # evacuate [128, 1024] in one instr; alternate ACT/DVE;
                # one full-row DMA per gt (HWDGE descriptor-gen is a serial
                # 625ns/DMA resource worth conserving)
                osl = obs[tt][:, half * 1024:(half + 1) * 1024]
                if half == 0:
                    nc.scalar.mul(out=osl, in_=ps, mul=OSC)
                else:
                    nc.vector.tensor_scalar_mul(out=osl, in0=ps, scalar1=OSC)
                    nc.sync.dma_start(
                        out_d[gt * 128:(gt + 1) * 128, :], obs[tt][:])

            def quantum_nck(tt, nck, pool):
                """Pool-rotation variant: one nck per quantum, [128, 512]
                psum tiles from the (tail-idle) proj/attention rings."""
                gt = q0 // 128 + tt
                if nck == 0:
                    obs[tt] = opool.tile([128, D], f16, tag="ob", name="ob")
                ps = ps_tile(pool)
                cproj_matmuls(ps, yc8, off, tt,
                              slice(nck * 512, (nck + 1) * 512))
                osl = obs[tt][:, nck * 512:(nck + 1) * 512]
                if nck % 2 == 0 and not dve_only:
                    nc.scalar.mul(out=osl, in_=ps, mul=OSC)
                else:
                    nc.vector.tensor_scalar_mul(out=osl, in0=ps, scalar1=OSC)
                if nck % 2 == 1:
                    # per-half DMAs: the tail has HWDGE to spare and the
                    # earlier transfer start shortens the final drain. With
                    # spread_dma the configs round-robin over the SP/ACT/DVE
                    # queues -- at the tail those SEQs are idle, so three
                    # DMAs dispatch concurrently instead of serializing on SP
                    eng = (nc.sync if not spread_dma
                           else (nc.sync, nc.scalar)[(2 * tt + nck // 2) % 2])
                    eng.dma_start(
                        out_d[gt * 128:(gt + 1) * 128,
                              (nck - 1) * 512:(nck + 1) * 512],
                        obs[tt][:, (nck - 1) * 512:(nck + 1) * 512])

            if pools:
                return [
                    (lambda tt=tt, nck=nck,
                     pool=pools[(4 * tt + nck) % len(pools)]:
                     quantum_nck(tt, nck, pool))
                    for tt in range(W // 128) for nck in range(4)
                ]
            return [
                (lambda tt=tt, half=half: quantum(tt, half, None))
                for tt in range(W // 128) for half in range(2)
            ]

        def drain(filler):
            filler = iter(filler) if not hasattr(filler, "__next__") else filler
            for q in filler:
                if q is not None:
                    q()

        def y_tiles(tag):
            yc = ypool.tile([128, H_LOC, TCH], f16, tag=tag, name="yc")
            yc8 = y8pool.tile([128, H_LOC, 2, TCH], f8, tag=tag + "8",
                              name="yc8")
            return yc, yc8

        # Emission order: a software pipeline whose backbone is the four
        # attention spans' exp streams chained back-to-back on ACT, with all
        # PE-dense work (projections, k/v, c_proj) hung inside the spans as
        # fillers. Chunk 0's attention runs FIRST -- everything it needs
        # exists by the time chunk 1's projections own PE, and its exp
        # stream fills ACT's otherwise-idle opening. c_proj(c) fills the
        # spans two chunks later (after wo8p lands); the only post-span tail
        # is c_proj(3).
        xc0 = issue_x(0)
        qc0, q80 = proj_q(0, xc0, do_rope=False)
        kv0 = kv_quanta(0, xc0)
        drain(kv0[:2])           # k heads; v halves become attn0 fillers
        kv0[4]()                 # rope(k0): tables ride in the chunk-0 batch
        rope_q(qc0, q80, 0)
        yc0, yc80 = y_tiles("yc0")
        xc1 = issue_x(1)
        kv1 = kv_quanta(1, xc1)
        pq1, qc1, q81 = proj_quanta(1, xc1)
        f0 = [kv0[2], kv0[3], pq1[0], kv1[0], pq1[1], kv1[1], pq1[2],
              kv1[2], kv1[3], kv1[4]]
        drain(attn_span(0, TCH, q80, 0, yc0, yc80, filler=f0, fill_per_jt=2))
        yc1, yc81 = y_tiles("yc")
        xc2 = issue_x(2)
        kv2 = kv_quanta(2, xc2)
        pq2, qc2, q82 = proj_quanta(2, xc2)
        f1 = [pq2[0], kv2[0], pq2[1], kv2[1], pq2[2], kv2[2], kv2[3], kv2[4]]
        drain(attn_span(TCH, TCH, q81, 0, yc1, yc81, filler=f1))
        yc2, yc82 = y_tiles("yc")
        xc3 = issue_x(3)
        kv3 = kv_quanta(3, xc3)
        pq3, qc3, q83 = proj_quanta(3, xc3)
        c0q = cproj_quanta(0, TCH, yc80, 0, pools=(psum_p,), dve_only=True)
        c1q = cproj_quanta(TCH, TCH, yc81, 0, pools=(psum_p,))
        f2 = [pq3[0], kv3[0], pq3[1], kv3[1], pq3[2], kv3[2], kv3[3], kv3[4]]
        f2 = f2 + [q for pair in zip(c0q, c1q) for q in pair]
        lf2 = attn_span(2 * TCH, TCH, q82, 0, yc2, yc82, filler=f2,
                        fill_per_jt=1)
        yc3, yc83 = y_tiles("yc")
        c2q = cproj_quanta(2 * TCH, TCH, yc82, 0, pools=(psum_p,))
        import itertools
        f3 = itertools.chain(lf2, c2q)
        drain(attn_span(3 * TCH, TCH, q83, 0, yc3, yc83, filler=f3,
                        fill_per_jt=4, fast_quant=True))
        drain(cproj_quanta(3 * TCH, TCH, yc83, 0, pools=(psum_ot, psum_p),
                           spread_dma=True))

    nc.compile()
    _CACHE["nc"] = nc
    return nc


def host_inputs(x, Wq, Wk, Wv, Wo):
    """Per-core input dicts (host-side shard + transpose + quantize + tables)."""
    import ml_dtypes

    f8 = ml_dtypes.float8_e4m3

    def pack_x8(a):  # (D, T) f32 -> (128, KO2, 2, T) fp8 DoubleRow layout
        return np.ascontiguousarray(
            a.reshape(KO2, 2, 128, T).transpose(2, 0, 1, 3)).astype(f8)

    x2 = np.ascontiguousarray(x.reshape(T, D).T).astype(np.float32)  # (D, T)
    x8 = pack_x8(x2)
    # fp8 residual (scaled 8x) for the v projection's second pass
    x8lo = pack_x8(
        8.0 * (x2 - x8.transpose(1, 2, 0, 3).reshape(D, T).astype(np.float32)))

    half = DH // 2  # 64
    af = (1.0 / 1024.0) ** np.linspace(0.0, 1.0, DH // 4, dtype=np.float32)
    af = np.concatenate([af, np.zeros(DH // 4, np.float32)])         # (64,)
    theta = np.arange(T, dtype=np.float32)[:, None] * af[None, :]    # (T, 64)
    cos = np.cos(theta).T.astype(np.float32)                         # (64, T)
    sin = np.sin(theta).T.astype(np.float32)
    ctab1 = np.concatenate([cos, cos], axis=0)                       # (128, T)
    stab1 = np.concatenate([sin, -sin], axis=0)
    # single-head; on-chip users broadcast over the head dim: (128, 1, T)
    ctab = ctab1[:, None, :].astype(np.float16)
    stab = stab1[:, None, :].astype(np.float16)

    roll = np.zeros((128, 128), np.float16)
    for p in range(128):
        roll[p, (p + half) % 128] = 1.0
    tri1 = np.triu(np.ones((128, 128), np.float16))  # tri[j, i] = i >= j
    tri = tri1[:, None, :]

    shared = {
        "x8": x8, "x8lo": x8lo, "ctab": ctab, "stab": stab,
        "roll": roll, "tri": tri,
    }

    def pack_pre(wt):  # pre-scaled (D, HD_LOC) f32 -> DoubleRow fp8 layout
        return np.ascontiguousarray(
            wt.reshape(KO2, 2, 128, HD_LOC).transpose(2, 0, 1, 3)).astype(f8)

    def pack_w8(w):  # (HD_LOC, D) slice -> (128, KO2, 2, HD_LOC) fp8, x WS
        return pack_pre((w.T * WS).astype(np.float32))

    in_maps = []
    for c in range(N_CORES):
        sl = slice(c * HD_LOC, (c + 1) * HD_LOC)
        wv_t = Wv[sl, :].T.astype(np.float32)               # (D, HD_LOC)
        wv8a = pack_pre(wv_t * WS)
        # residual of the 64x-quantized Wv, itself scaled 64x
        wv_res = wv_t - wv8a.transpose(1, 2, 0, 3).reshape(D, HD_LOC).astype(
            np.float32) / WS
        # c_proj weights: BETA-prescaled fp8 main plane (duplicated in the
        # DR pair slots) + fp8 residual plane, laid out [p, h, (2,) D] with
        # p = dh-within-head to match the yc8 partition mapping
        wol = np.ascontiguousarray(
            (Wo[:, sl].astype(np.float32) * (BETA / 3.0)).T)  # (HD_LOC, D)
        w8p = wol.astype(f8)
        wres8 = (wol - w8p.astype(np.float32)).astype(f8)
        wo8p = np.ascontiguousarray(
            w8p.reshape(H_LOC, 128, D).transpose(1, 0, 2))   # (128, h, D)
        wr8 = np.ascontiguousarray(
            wres8.reshape(H_LOC, 128, D).transpose(1, 0, 2))
        in_maps.append({
            **shared,
            "wq8": pack_w8(Wq[sl, :]),
            "wk8": pack_w8(Wk[sl, :]),
            "wv8a": wv8a,
            "wv8b": pack_pre(wv_t * 8.0),
            "wv8c": pack_pre(wv_res * WS),
            "wo8p": wo8p,
            "wr8": wr8,
        })
    return in_maps


def _get_runner():
    """Build the program + a persistent jitted SPMD executable (once)."""
    if "runner" in _CACHE:
        return _CACHE["runner"]

    import jax
    import concourse.mybir as mybir
    from concourse.bass2jax import (
        _bass_exec_p,
        install_neuronx_cc_hook,
        partition_id_tensor,
    )
    from jax.experimental.shard_map import shard_map
    from jax.sharding import Mesh, PartitionSpec

    nc = build_program()
    install_neuronx_cc_hook()
    assert nc.dbg_addr is None
    pid_name = nc.partition_id_tensor.name if nc.partition_id_tensor else None

    in_names, out_names, out_avals, zero_outs = [], [], [], []
    for alloc in nc.m.functions[0].allocations:
        if not isinstance(alloc, mybir.MemoryLocationSet):
            continue
        name = alloc.memorylocations[0].name
        if alloc.kind == "ExternalInput":
            if name != pid_name:
                in_names.append(name)
        elif alloc.kind == "ExternalOutput":
            out_names.append(name)
            shape = tuple(alloc.tensor_shape)
            dtype = mybir.dt.np(alloc.dtype)
            out_avals.append(jax.core.ShapedArray(shape, dtype))
            zero_outs.append(np.zeros(shape, dtype))
    n_params = len(in_names)
    all_names = list(in_names) + list(out_names)
    if pid_name is not None:
        all_names.append(pid_name)
    donate = tuple(range(n_params, n_params + len(out_names)))

    def _body(*args):
        operands = list(args)
        if pid_name is not None:
            operands.append(partition_id_tensor())
        outs = _bass_exec_p.bind(
            *operands,
            out_avals=tuple(out_avals),
            in_names=tuple(all_names),
            out_names=tuple(out_names),
            lowering_input_output_aliases=(),
            sim_require_finite=True,
            sim_require_nnan=True,
            nc=nc,
        )
        return tuple(outs)

    devices = jax.devices()[:N_CORES]
    mesh = Mesh(np.asarray(devices), ("core",))
    in_specs = (PartitionSpec("core"),) * (n_params + len(out_names))
    out_specs = (PartitionSpec("core"),) * len(out_names)
    fn = jax.jit(
        shard_map(_body, mesh=mesh, in_specs=in_specs, out_specs=out_specs,
                  check_rep=False),
        donate_argnums=donate,
        keep_unused=True,
    )
    runner = (fn, in_names, out_names, out_avals, zero_outs)
    _CACHE["runner"] = runner
    return runner


def run_spmd(in_maps):
    """Execute the SPMD program; returns per-core output dicts."""
    fn, in_names, out_names, out_avals, zero_outs = _get_runner()
    concat_in = [
        np.concatenate([np.asarray(in_maps[c][n]) for c in range(N_CORES)], axis=0)
        for n in in_names
    ]
    concat_zeros = [
        np.zeros((N_CORES * z.shape[0], *z.shape[1:]), z.dtype) for z in zero_outs
    ]
    out_arrs = fn(*concat_in, *concat_zeros)
    return [
        {n: np.asarray(out_arrs[i]).reshape(N_CORES, *out_avals[i].shape)[c]
         for i, n in enumerate(out_names)}
        for c in range(N_CORES)
    ]


def kernel(x, Wq, Wk, Wv, Wo):
    in_maps = host_inputs(np.asarray(x), np.asarray(Wq), np.asarray(Wk),
                          np.asarray(Wv), np.asarray(Wo))
    results = run_spmd(in_maps)
    out = results[0]["outp"].astype(np.float64)
    for c in range(1, N_CORES):
        out += results[c]["outp"].astype(np.float64)
    return out.astype(np.float32).reshape(1, T, D)
